# revision 15
# baseline (speedup 1.0000x reference)
"""Causal multi-head attention on 8 Trainium2 NeuronCores.

Sharding: core c -> batch (c // 4), head-group (c % 4) of 4 heads
(tensor-parallel over the 16 heads, data-parallel over batch=2).
Each core computes its 4 heads' contribution to the output projection;
the host sums the 4 per-head-group partials per batch (the "all-reduce")
and adds b_O.

Kernel layout (per core):
  - transposed [feature, seq] layout throughout; softmax reduction over
    keys lands on PSUM partitions and is done with a ones-matmul on PE.
  - scores for the 2 heads of a pair co-execute via row-groups (K=64
    each); PV and the denominator co-execute via column groups.
  - single fine-grained emission loop: attention chunks are emitted
    with one-chunk lookahead (scores(c+1) before PV(c)) and "filler"
    matmul units (QK/V projections of later tiles, W_O contraction of
    finished query tiles) are drained into the PE stream between
    chunks, sized by an ACT-vs-PE credit model, so the PE never idles
    while the scalar engine computes exp.
  - diagonal score tiles are packed: head1's columns start right after
    head0's causal-trimmed width, shrinking the exp instruction.
  - output DMAs issue from the sync queue (HWDGE) so the scalar engine
    runs exp back-to-back; output is stored bf16 (host sums in fp32).
"""

import os
import sys

for _p in ("/opt/trn_rl_repo", "/root/.axon_site/_ro/trn_rl_repo"):
    if os.path.isdir(_p) and _p not in sys.path:
        sys.path.append(_p)

import ml_dtypes
import numpy as np

import concourse.bacc as bacc
import concourse.mybir as mybir
import concourse.tile as tile
from concourse.bass_utils import run_bass_kernel_spmd

F32 = mybir.dt.float32
BF16 = mybir.dt.bfloat16

B = 2          # batch
S = 2048       # sequence length
DM = 1024      # d_model
DH = 64        # d_head
NHEAD = 16     # total heads
NH = 4         # heads per core
NPAIR = 2      # head pairs per core
DC = DM // 128   # d_model chunks of 128 -> 8
KC = S // 128    # key chunks of 128 -> 16
QT = S // 512    # query tiles of 512 -> 4

# Set by test harness to capture HW profile; harmless defaults for grading.
TRACE = False
TRACE_DIR = None
LAST_EXEC_NS = None


def _build(with_bias: bool):
    nc = bacc.Bacc("TRN2", target_bir_lowering=False, debug=False)

    xT = nc.dram_tensor("xT", [128, DC, S], BF16, kind="ExternalInput").ap()
    # wq/wk packed pair-major: [128, NPAIR, DC, 128]
    wq = nc.dram_tensor("wq", [128, NPAIR * DC * 128], BF16, kind="ExternalInput").ap()
    wk = nc.dram_tensor("wk", [128, NPAIR * DC * 128], BF16, kind="ExternalInput").ap()
    wv = nc.dram_tensor("wv", [128, DC * NH * DH], BF16, kind="ExternalInput").ap()
    wo = nc.dram_tensor("wo", [128, NPAIR * DM], BF16, kind="ExternalInput").ap()
    mask = nc.dram_tensor("mask", [128, 128], BF16, kind="ExternalInput").ap()
    if with_bias:
        bq = nc.dram_tensor("bq", [1, NH * DH], BF16, kind="ExternalInput").ap()
        bk = nc.dram_tensor("bk", [1, NH * DH], BF16, kind="ExternalInput").ap()
        bv = nc.dram_tensor("bv", [1, NH * DH], BF16, kind="ExternalInput").ap()
    outT = nc.dram_tensor("outT", [128, DC, S], BF16, kind="ExternalOutput").ap()

    with tile.TileContext(nc) as tc:
        with (
            tc.tile_pool(name="const", bufs=1) as cpool,
            tc.tile_pool(name="qk", bufs=1) as qkpool,
            tc.tile_pool(name="xt", bufs=8) as xtpool,
            tc.tile_pool(name="expS", bufs=4) as epool,
            tc.tile_pool(name="small", bufs=2) as spool,
            tc.tile_pool(name="zt", bufs=8) as ztpool,
            tc.tile_pool(name="out", bufs=3) as opool,
            tc.tile_pool(name="ps", bufs=1, space="PSUM") as psP,
        ):
            wq_sb = cpool.tile([128, NPAIR, DC, 128], BF16, name="wq")
            wk_sb = cpool.tile([128, NPAIR, DC, 128], BF16, name="wk")
            wv_sb = cpool.tile([128, DC, NH * DH], BF16, name="wv")
            wo_sb = cpool.tile([128, NPAIR, DM], BF16, name="wo")
            mask_sb = cpool.tile([128, 128], BF16, name="mask")
            ones_bf = cpool.tile([128, DH], BF16, name="ones_bf")
            nc.vector.memset(ones_bf[:, :], 1.0)
            # warm up the PE HAM clock-gate during the input-DMA wait so
            # the first real matmuls run at 2.4 GHz instead of 1.2.
            warm = psP.tile([128, 512], F32, name="ps_fill", bufs=2)
            for _ in range(40):
                nc.tensor.matmul(
                    warm[0:64, 0:64], lhsT=ones_bf[:, :], rhs=ones_bf[:, :],
                    start=True, stop=True,
                )
            if with_bias:
                ones32 = cpool.tile([128, 512], BF16, name="ones32")
                nc.vector.memset(ones32[:, :], 1.0)
                bq_sb = cpool.tile([128, NH * DH], BF16, name="bq")
                bk_sb = cpool.tile([128, NH * DH], BF16, name="bk")
                bv_sb = cpool.tile([128, NH * DH], BF16, name="bv")

            qt_sb = [qkpool.tile([128, S], BF16, name=f"qt{p}") for p in range(NPAIR)]
            kt_sb = [qkpool.tile([128, S], BF16, name=f"kt{p}") for p in range(NPAIR)]
            v_sb = qkpool.tile([128, KC, NH * DH], BF16, name="v")
            xt_sb = xtpool.tile([128, DC, S], BF16, name="xt", bufs=1)

            # ---- input DMAs (sync queue / HWDGE), ordered so the pair-0
            # Q/K projection of seq-tile 0 and the first V chunks can
            # start as early as possible.
            def dma_x_tile(t):
                nc.sync.dma_start(
                    xt_sb[:, :, t * 512:(t + 1) * 512],
                    xT[:, :, t * 512:(t + 1) * 512],
                )

            nc.sync.dma_start(wq_sb[:, 0, :, :], wq[:, 0:DC * 128])
            nc.sync.dma_start(wk_sb[:, 0, :, :], wk[:, 0:DC * 128])
            dma_x_tile(0)
            nc.sync.dma_start(wv_sb[:, :, :], wv[:, :])
            nc.sync.dma_start(mask_sb[:, :], mask[:, :])
            dma_x_tile(1)
            nc.sync.dma_start(wq_sb[:, 1, :, :], wq[:, DC * 128:])
            nc.sync.dma_start(wk_sb[:, 1, :, :], wk[:, DC * 128:])
            dma_x_tile(2)
            dma_x_tile(3)
            nc.sync.dma_start(wo_sb[:, :, :], wo[:, :])
            if with_bias:
                nc.sync.dma_start(bq_sb[0:1, :], bq[:, :])
                nc.sync.dma_start(bk_sb[0:1, :], bk[:, :])
                nc.sync.dma_start(bv_sb[0:1, :], bv[:, :])

            # ---------------- filler units ----------------
            # Each unit is (tag, pe_ns_estimate, closure). Emitted into
            # the instruction stream between attention chunks.
            def qk_unit(p, pj, q):
                def emit():
                    w_sb = wq_sb if pj == 0 else wk_sb
                    ps = psP.tile([128, 512], F32, name="ps_fill", bufs=2)
                    for c in range(DC):
                        nc.tensor.matmul(
                            ps[:, :],
                            lhsT=w_sb[:, p, c, :],
                            rhs=xt_sb[:, c, q * 512:(q + 1) * 512],
                            start=(c == 0),
                            stop=(c == DC - 1 and not with_bias),
                        )
                    if with_bias:
                        bias_t = bq_sb if pj == 0 else bk_sb
                        nc.tensor.matmul(
                            ps[:, :],
                            lhsT=bias_t[0:1, p * 128:(p + 1) * 128],
                            rhs=ones32[0:1, :],
                            start=False,
                            stop=True,
                        )
                    dst = qt_sb[p] if pj == 0 else kt_sb[p]
                    nc.vector.tensor_copy(dst[:, q * 512:(q + 1) * 512], ps[:, :])
                return (f"{'qk'[pj]}{p}t{q}", 1750, emit)

            def v_unit(k):
                def emit():
                    ps = psP.tile([128, 512], F32, name="ps_fill", bufs=2)
                    for c in range(DC):
                        nc.tensor.matmul(
                            ps[:, :NH * DH],
                            lhsT=xt_sb[:, c, k * 128:(k + 1) * 128],
                            rhs=wv_sb[:, c, :],
                            start=(c == 0),
                            stop=(c == DC - 1 and not with_bias),
                            skip_group_check=True,
                        )
                    if with_bias:
                        nc.tensor.matmul(
                            ps[:, :NH * DH],
                            lhsT=ones32[0:1, 0:128],
                            rhs=bv_sb[0:1, :],
                            start=False,
                            stop=True,
                            skip_group_check=True,
                        )
                    nc.vector.tensor_copy(v_sb[:, k, :], ps[:, :NH * DH])
                return (f"v{k}", 900, emit)

            zts = {}  # (p, j) -> zt tile
            ot_tiles = {}
            spill_mode = [False]
            wo_alt = [0]

            def wo_unit(j, d):
                def emit():
                    # in the post-attention spill phase the scores pool is
                    # idle; alternate onto it for deeper psum pipelining.
                    wo_alt[0] ^= 1
                    if spill_mode[0] and wo_alt[0]:
                        ps = psP.tile([128, 1024], F32, name="ps_sc", bufs=2)
                    else:
                        ps = psP.tile([128, 512], F32, name="ps_fill", bufs=2)
                    for p in range(NPAIR):
                        nc.tensor.matmul(
                            ps[:, 0:512],
                            lhsT=wo_sb[:, p, d * 128:(d + 1) * 128],
                            rhs=zts[(p, j)][:, :],
                            start=(p == 0),
                            stop=(p == NPAIR - 1),
                        )
                    if j not in ot_tiles:
                        ot_tiles[j] = opool.tile([128, DC, 512], BF16, name="ot", bufs=2)
                    ot = ot_tiles[j]
                    nc.vector.tensor_copy(ot[:, d, :], ps[:, 0:512])
                    if d == DC - 1:
                        # one batched DMA per query tile
                        nc.sync.dma_start(
                            outT[:, :, j * 512:(j + 1) * 512], ot[:, :, :],
                        )
                return (f"wo{j}d{d}", 500, emit)

            filler = []
            for p in range(NPAIR):
                for q in range(QT):
                    filler.append(qk_unit(p, 0, q))
                    filler.append(qk_unit(p, 1, q))
                    if p == 0:
                        for k in range(4 * q, 4 * q + 4):
                            filler.append(v_unit(k))
            done_tags = set()

            # credit[0]: cumulative ACT-work minus PE-work emitted; every
            # drained unit debits it so drains spread instead of clumping.
            credit = [0.0]

            drained = [0]

            def drain_one():
                tag, cost, emit = filler.pop(0)
                emit()
                done_tags.add(tag)
                credit[0] -= cost
                drained[0] += 1
                return cost

            def need(tag):
                while tag not in done_tags:
                    drain_one()

            def drain_credit():
                while filler and credit[0] >= filler[0][1]:
                    drain_one()

            def emit_scores(p, j, c):
                a = max(0, 128 * (c - 4 * j))
                need(f"q{p}t{j}")
                need(f"k{p}t{c // 4}")
                ps = psP.tile([128, 1024], F32, name="ps_sc", bufs=2)
                nc.tensor.matmul(
                    ps[:, a:512],
                    lhsT=kt_sb[p][0:64, c * 128:(c + 1) * 128],
                    rhs=qt_sb[p][0:64, j * 512 + a:(j + 1) * 512],
                    start=True,
                    stop=True,
                )
                nc.tensor.matmul(
                    ps[:, 512:1024 - a],
                    lhsT=kt_sb[p][64:128, c * 128:(c + 1) * 128],
                    rhs=qt_sb[p][64:128, j * 512 + a:(j + 1) * 512],
                    start=True,
                    stop=True,
                )
                es = epool.tile([128, 1024], BF16, name="es", bufs=4)
                nc.scalar.activation(
                    es[:, a:1024 - a],
                    ps[:, a:1024 - a],
                    mybir.ActivationFunctionType.Exp,
                )
                if a or c == 4 * j:
                    nc.vector.tensor_mul(
                        out=es[:, a:a + 128], in0=es[:, a:a + 128], in1=mask_sb[:, :],
                    )
                    nc.vector.tensor_mul(
                        out=es[:, 512:640], in0=es[:, 512:640], in1=mask_sb[:, :],
                    )
                credit[0] += ((1024 - 2 * a) + 352) / 1.2 - (2 * (512 - a) / 2.4 + 60)
                return es, a

            def emit_pv(p, j, c, nck, es, a, ps_z, ps_s):
                need(f"v{c}")
                for hi in range(2):
                    col = 64 * hi
                    hcore = 2 * p + hi
                    rhs = es[:, a:512] if hi == 0 else es[:, 512:1024 - a]
                    nc.tensor.matmul(
                        ps_z[col:col + 64, a:512],
                        lhsT=v_sb[:, c, hcore * DH:(hcore + 1) * DH],
                        rhs=rhs,
                        start=(c == 0),
                        stop=(c == nck - 1),
                        tile_position=(0, col),
                        skip_group_check=True,
                    )
                for hi in range(2):
                    col = 64 * hi
                    rhs = es[:, a:512] if hi == 0 else es[:, 512:1024 - a]
                    nc.tensor.matmul(
                        ps_s[col:col + 64, a:512],
                        lhsT=ones_bf[:, :],
                        rhs=rhs,
                        start=(c == 0),
                        stop=(c == nck - 1),
                        tile_position=(0, col),
                        skip_group_check=True,
                    )
                credit[0] -= 4 * (512 - a) / 2.4 + 120

            def finish_iter(p, j, ps_z, ps_s):
                recip = spool.tile([128, 512], F32, name="recip")
                nc.vector.reciprocal_approx_fast(recip[:, :], ps_s[:, :])
                zt = ztpool.tile([128, 512], BF16, name="zt")
                nc.vector.tensor_mul(zt[:, :], ps_z[:, :], recip[:, :])
                zts[(p, j)] = zt
                if p == 1:
                    for d in range(DC):
                        filler.append(wo_unit(j, d))

            # flat global chunk pipeline across all (p, j) iterations;
            # scores run one chunk ahead of PV so exp always overlaps.
            # Filler units needed by iteration i+1 are drained evenly
            # across iteration i's chunks (deadline quota); extra units
            # drain on banked ACT-slack credit.
            iters = [(p, j) for p in range(NPAIR) for j in range(QT)]
            stream = []
            for i, (p, j) in enumerate(iters):
                for c in range(4 * (j + 1)):
                    stream.append((i, p, j, c))

            def due_len(i):
                if i + 1 >= len(iters):
                    return 0
                p1, j1 = iters[i + 1]
                tags = {f"q{p1}t{j1}", f"k{p1}t{j1}"}
                tags |= {f"v{k}" for k in range(4 * (j1 + 1))}
                pos = [k for k, (tag, _, _) in enumerate(filler) if tag in tags]
                return (max(pos) + 1) if pos else 0

            prev = None
            it_state = {}
            for g, (i, p, j, c) in enumerate(stream):
                nck = 4 * (j + 1)
                if c == 0:
                    it_state[i] = [due_len(i), drained[0]]  # [due, base]
                es, a = emit_scores(p, j, c)
                st = it_state[i]
                want = -(-st[0] * (c + 1) // nck)  # ceil
                while filler and drained[0] - st[1] < want:
                    drain_one()
                drain_credit()
                if prev is not None:
                    pi, pp, pj, pc, pes, pa, p_z, p_s = prev
                    emit_pv(pp, pj, pc, 4 * (pj + 1), pes, pa, p_z, p_s)
                    if pc == 4 * (pj + 1) - 1:
                        finish_iter(pp, pj, p_z, p_s)
                if c == 0:
                    ps_z = psP.tile([128, 512], F32, name="ps_z", bufs=1)
                    ps_s = psP.tile([128, 512], F32, name="ps_s", bufs=1)
                prev = (i, p, j, c, es, a, ps_z, ps_s)
            pi, pp, pj, pc, pes, pa, p_z, p_s = prev
            emit_pv(pp, pj, pc, 4 * (pj + 1), pes, pa, p_z, p_s)
            finish_iter(pp, pj, p_z, p_s)
            spill_mode[0] = True
            while filler:
                drain_one()

    nc.compile()
    return nc


_cache = {}


def _get(with_bias: bool):
    if with_bias not in _cache:
        _cache[with_bias] = _build(with_bias)
    return _cache[with_bias]


def kernel(x, W_Q, W_K, W_V, W_O, b_Q, b_K, b_V, b_O):
    global LAST_EXEC_NS
    x = np.asarray(x, dtype=np.float32)
    W_Q = np.asarray(W_Q, dtype=np.float32)
    W_K = np.asarray(W_K, dtype=np.float32)
    W_V = np.asarray(W_V, dtype=np.float32)
    W_O = np.asarray(W_O, dtype=np.float32)
    b_Q = np.asarray(b_Q, dtype=np.float32)
    b_K = np.asarray(b_K, dtype=np.float32)
    b_V = np.asarray(b_V, dtype=np.float32)
    b_O = np.asarray(b_O, dtype=np.float32)

    with_bias = bool(np.any(b_Q) or np.any(b_K) or np.any(b_V))
    nc = _get(with_bias)

    xT = np.ascontiguousarray(x.transpose(0, 2, 1))  # [B, DM, S]
    kp = np.arange(128)[:, None]
    qf = np.arange(128)[None, :]
    mask = np.where(qf >= kp, 1.0, 0.0).astype(ml_dtypes.bfloat16)

    in_maps = []
    for core in range(8):
        b, g = divmod(core, 4)
        hs = slice(NH * g, NH * g + NH)
        bf = ml_dtypes.bfloat16

        def packqk(w):  # [DM, NH*DH] -> [128, NPAIR, DC, 128] pair-major
            return np.ascontiguousarray(
                w.reshape(DC, 128, NPAIR, 128).transpose(1, 2, 0, 3)
                .reshape(128, NPAIR * DC * 128)
            )

        def packw(w):  # [DM, NH*DH] -> [128, DC*NH*DH] chunk-major
            return np.ascontiguousarray(
                w.reshape(DC, 128, NH * DH).transpose(1, 0, 2).reshape(128, DC * NH * DH)
            )

        m = {
            "xT": np.ascontiguousarray(xT[b].reshape(DC, 128, S).transpose(1, 0, 2)).astype(bf),
            "wq": packqk((W_Q[hs] * 0.125).transpose(1, 0, 2).reshape(DM, NH * DH).astype(bf)),
            "wk": packqk(W_K[hs].transpose(1, 0, 2).reshape(DM, NH * DH).astype(bf)),
            "wv": packw(W_V[hs].transpose(1, 0, 2).reshape(DM, NH * DH).astype(bf)),
            "wo": np.ascontiguousarray(
                W_O[hs].reshape(NH * DH, DM).astype(bf)
                .reshape(NPAIR, 128, DM).transpose(1, 0, 2).reshape(128, NPAIR * DM)
            ),
            "mask": mask,
        }
        if with_bias:
            m["bq"] = (b_Q[hs] * 0.125).reshape(1, NH * DH).astype(bf)
            m["bk"] = b_K[hs].reshape(1, NH * DH).astype(bf)
            m["bv"] = b_V[hs].reshape(1, NH * DH).astype(bf)
        in_maps.append(m)

    kwargs = {}
    if TRACE:
        kwargs = {"trace": True}
        if TRACE_DIR:
            kwargs["tmpdir"] = TRACE_DIR
    res = run_bass_kernel_spmd(nc, in_maps, list(range(8)), **kwargs)
    LAST_EXEC_NS = res.exec_time_ns

    out = np.empty((B, S, DM), dtype=np.float32)
    for b in range(B):
        acc = res.results[4 * b]["outT"].astype(np.float32)
        for g in range(1, 4):
            acc = acc + res.results[4 * b + g]["outT"].astype(np.float32)
        # [128, DC, S] -> [DM, S]
        full = acc.transpose(1, 0, 2).reshape(DM, S)
        out[b] = full.T + b_O[None, :]
    return out


# revision 16
# speedup vs baseline: 1.0114x; 1.0114x over previous
"""Causal multi-head attention on 8 Trainium2 NeuronCores.

Sharding: core c -> batch (c // 4), head-group (c % 4) of 4 heads
(tensor-parallel over the 16 heads, data-parallel over batch=2).
Each core computes its 4 heads' contribution to the output projection;
the host sums the 4 per-head-group partials per batch (the "all-reduce")
and adds b_O.

Kernel layout (per core):
  - transposed [feature, seq] layout throughout; softmax reduction over
    keys lands on PSUM partitions and is done with a ones-matmul on PE.
  - scores for the 2 heads of a pair co-execute via row-groups (K=64
    each); PV and the denominator co-execute via column groups.
  - single fine-grained emission loop: attention chunks are emitted
    with one-chunk lookahead (scores(c+1) before PV(c)) and "filler"
    matmul units (QK/V projections of later tiles, W_O contraction of
    finished query tiles) are drained into the PE stream between
    chunks, sized by an ACT-vs-PE credit model, so the PE never idles
    while the scalar engine computes exp.
  - diagonal score tiles are packed: head1's columns start right after
    head0's causal-trimmed width, shrinking the exp instruction.
  - output DMAs issue from the sync queue (HWDGE) so the scalar engine
    runs exp back-to-back; output is stored bf16 (host sums in fp32).
"""

import os
import sys

for _p in ("/opt/trn_rl_repo", "/root/.axon_site/_ro/trn_rl_repo"):
    if os.path.isdir(_p) and _p not in sys.path:
        sys.path.append(_p)

import ml_dtypes
import numpy as np

import concourse.bacc as bacc
import concourse.mybir as mybir
import concourse.tile as tile
from concourse.bass_utils import run_bass_kernel_spmd

F32 = mybir.dt.float32
BF16 = mybir.dt.bfloat16

B = 2          # batch
S = 2048       # sequence length
DM = 1024      # d_model
DH = 64        # d_head
NHEAD = 16     # total heads
NH = 4         # heads per core
NPAIR = 2      # head pairs per core
DC = DM // 128   # d_model chunks of 128 -> 8
KC = S // 128    # key chunks of 128 -> 16
QT = S // 512    # query tiles of 512 -> 4

# Set by test harness to capture HW profile; harmless defaults for grading.
TRACE = False
TRACE_DIR = None
LAST_EXEC_NS = None


def _build(with_bias: bool):
    nc = bacc.Bacc("TRN2", target_bir_lowering=False, debug=False)

    xT = nc.dram_tensor("xT", [DM, S], BF16, kind="ExternalInput").ap()
    # wq/wk packed pair-major: [128, NPAIR, DC, 128]
    wq = nc.dram_tensor("wq", [128, NPAIR * DC * 128], BF16, kind="ExternalInput").ap()
    wk = nc.dram_tensor("wk", [128, NPAIR * DC * 128], BF16, kind="ExternalInput").ap()
    wv = nc.dram_tensor("wv", [128, DC * NH * DH], BF16, kind="ExternalInput").ap()
    wo = nc.dram_tensor("wo", [128, NPAIR * DM], BF16, kind="ExternalInput").ap()
    mask = nc.dram_tensor("mask", [128, 128], BF16, kind="ExternalInput").ap()
    if with_bias:
        bq = nc.dram_tensor("bq", [1, NH * DH], BF16, kind="ExternalInput").ap()
        bk = nc.dram_tensor("bk", [1, NH * DH], BF16, kind="ExternalInput").ap()
        bv = nc.dram_tensor("bv", [1, NH * DH], BF16, kind="ExternalInput").ap()
    outT = nc.dram_tensor("outT", [128, DC, S], BF16, kind="ExternalOutput").ap()

    with tile.TileContext(nc) as tc:
        with (
            tc.tile_pool(name="const", bufs=1) as cpool,
            tc.tile_pool(name="qk", bufs=1) as qkpool,
            tc.tile_pool(name="xt", bufs=8) as xtpool,
            tc.tile_pool(name="expS", bufs=4) as epool,
            tc.tile_pool(name="small", bufs=2) as spool,
            tc.tile_pool(name="zt", bufs=8) as ztpool,
            tc.tile_pool(name="out", bufs=3) as opool,
            tc.tile_pool(name="ps", bufs=1, space="PSUM") as psP,
        ):
            wq_sb = cpool.tile([128, NPAIR, DC, 128], BF16, name="wq")
            wk_sb = cpool.tile([128, NPAIR, DC, 128], BF16, name="wk")
            wv_sb = cpool.tile([128, DC, NH * DH], BF16, name="wv")
            wo_sb = cpool.tile([128, NPAIR, DM], BF16, name="wo")
            mask_sb = cpool.tile([128, 128], BF16, name="mask")
            ones_bf = cpool.tile([128, DH], BF16, name="ones_bf")
            nc.vector.memset(ones_bf[:, :], 1.0)
            # warm up the PE HAM clock-gate during the input-DMA wait so
            # the first real matmuls run at 2.4 GHz instead of 1.2.
            warm = psP.tile([128, 512], F32, name="ps_fill", bufs=2)
            for _ in range(40):
                nc.tensor.matmul(
                    warm[0:64, 0:64], lhsT=ones_bf[:, :], rhs=ones_bf[:, :],
                    start=True, stop=True,
                )
            if with_bias:
                ones32 = cpool.tile([128, 512], BF16, name="ones32")
                nc.vector.memset(ones32[:, :], 1.0)
                bq_sb = cpool.tile([128, NH * DH], BF16, name="bq")
                bk_sb = cpool.tile([128, NH * DH], BF16, name="bk")
                bv_sb = cpool.tile([128, NH * DH], BF16, name="bv")

            qt_sb = [qkpool.tile([128, S], BF16, name=f"qt{p}") for p in range(NPAIR)]
            kt_sb = [qkpool.tile([128, S], BF16, name=f"kt{p}") for p in range(NPAIR)]
            v_sb = qkpool.tile([128, KC, NH * DH], BF16, name="v")
            xt = [xtpool.tile([128, S], BF16, name="xt") for _ in range(DC)]

            # ---- input DMAs (sync queue / HWDGE), ordered so the pair-0
            # Q/K projection of seq-tile 0 and the first V chunks can
            # start as early as possible.
            def dma_x_tile(t):
                for c in range(DC):
                    nc.sync.dma_start(
                        xt[c][:, t * 512:(t + 1) * 512],
                        xT[c * 128:(c + 1) * 128, t * 512:(t + 1) * 512],
                    )

            nc.sync.dma_start(wq_sb[:, 0, :, :], wq[:, 0:DC * 128])
            nc.sync.dma_start(wk_sb[:, 0, :, :], wk[:, 0:DC * 128])
            dma_x_tile(0)
            nc.sync.dma_start(wv_sb[:, :, :], wv[:, :])
            nc.sync.dma_start(mask_sb[:, :], mask[:, :])
            dma_x_tile(1)
            nc.sync.dma_start(wq_sb[:, 1, :, :], wq[:, DC * 128:])
            nc.sync.dma_start(wk_sb[:, 1, :, :], wk[:, DC * 128:])
            dma_x_tile(2)
            dma_x_tile(3)
            nc.sync.dma_start(wo_sb[:, :, :], wo[:, :])
            if with_bias:
                nc.sync.dma_start(bq_sb[0:1, :], bq[:, :])
                nc.sync.dma_start(bk_sb[0:1, :], bk[:, :])
                nc.sync.dma_start(bv_sb[0:1, :], bv[:, :])

            # ---------------- filler units ----------------
            # Each unit is (tag, pe_ns_estimate, closure). Emitted into
            # the instruction stream between attention chunks.
            def qk_unit(p, pj, q):
                def emit():
                    w_sb = wq_sb if pj == 0 else wk_sb
                    ps = psP.tile([128, 512], F32, name="ps_fill", bufs=2)
                    for c in range(DC):
                        nc.tensor.matmul(
                            ps[:, :],
                            lhsT=w_sb[:, p, c, :],
                            rhs=xt[c][:, q * 512:(q + 1) * 512],
                            start=(c == 0),
                            stop=(c == DC - 1 and not with_bias),
                        )
                    if with_bias:
                        bias_t = bq_sb if pj == 0 else bk_sb
                        nc.tensor.matmul(
                            ps[:, :],
                            lhsT=bias_t[0:1, p * 128:(p + 1) * 128],
                            rhs=ones32[0:1, :],
                            start=False,
                            stop=True,
                        )
                    dst = qt_sb[p] if pj == 0 else kt_sb[p]
                    nc.vector.tensor_copy(dst[:, q * 512:(q + 1) * 512], ps[:, :])
                return (f"{'qk'[pj]}{p}t{q}", 1750, emit)

            def v_unit(k):
                def emit():
                    ps = psP.tile([128, 512], F32, name="ps_fill", bufs=2)
                    for c in range(DC):
                        nc.tensor.matmul(
                            ps[:, :NH * DH],
                            lhsT=xt[c][:, k * 128:(k + 1) * 128],
                            rhs=wv_sb[:, c, :],
                            start=(c == 0),
                            stop=(c == DC - 1 and not with_bias),
                            skip_group_check=True,
                        )
                    if with_bias:
                        nc.tensor.matmul(
                            ps[:, :NH * DH],
                            lhsT=ones32[0:1, 0:128],
                            rhs=bv_sb[0:1, :],
                            start=False,
                            stop=True,
                            skip_group_check=True,
                        )
                    nc.vector.tensor_copy(v_sb[:, k, :], ps[:, :NH * DH])
                return (f"v{k}", 900, emit)

            zts = {}  # (p, j) -> zt tile
            ot_tiles = {}
            spill_mode = [False]
            wo_alt = [0]

            def wo_unit(j, d):
                def emit():
                    # in the post-attention spill phase the scores pool is
                    # idle; alternate onto it for deeper psum pipelining.
                    wo_alt[0] ^= 1
                    if spill_mode[0] and wo_alt[0]:
                        ps = psP.tile([128, 1024], F32, name="ps_sc", bufs=2)
                    else:
                        ps = psP.tile([128, 512], F32, name="ps_fill", bufs=2)
                    for p in range(NPAIR):
                        nc.tensor.matmul(
                            ps[:, 0:512],
                            lhsT=wo_sb[:, p, d * 128:(d + 1) * 128],
                            rhs=zts[(p, j)][:, :],
                            start=(p == 0),
                            stop=(p == NPAIR - 1),
                        )
                    if j not in ot_tiles:
                        ot_tiles[j] = opool.tile([128, DC, 512], BF16, name="ot", bufs=2)
                    ot = ot_tiles[j]
                    nc.vector.tensor_copy(ot[:, d, :], ps[:, 0:512])
                    if d == DC - 1:
                        # one batched DMA per query tile
                        nc.sync.dma_start(
                            outT[:, :, j * 512:(j + 1) * 512], ot[:, :, :],
                        )
                return (f"wo{j}d{d}", 500, emit)

            filler = []
            for p in range(NPAIR):
                for q in range(QT):
                    filler.append(qk_unit(p, 0, q))
                    filler.append(qk_unit(p, 1, q))
                    if p == 0:
                        for k in range(4 * q, 4 * q + 4):
                            filler.append(v_unit(k))
            done_tags = set()

            # credit[0]: cumulative ACT-work minus PE-work emitted; every
            # drained unit debits it so drains spread instead of clumping.
            credit = [0.0]

            drained = [0]

            def drain_one():
                tag, cost, emit = filler.pop(0)
                emit()
                done_tags.add(tag)
                credit[0] -= cost
                drained[0] += 1
                return cost

            def need(tag):
                while tag not in done_tags:
                    drain_one()

            def drain_credit():
                while filler and credit[0] >= filler[0][1]:
                    drain_one()

            def emit_scores(p, j, c):
                a = max(0, 128 * (c - 4 * j))
                need(f"q{p}t{j}")
                need(f"k{p}t{c // 4}")
                ps = psP.tile([128, 1024], F32, name="ps_sc", bufs=2)
                nc.tensor.matmul(
                    ps[:, a:512],
                    lhsT=kt_sb[p][0:64, c * 128:(c + 1) * 128],
                    rhs=qt_sb[p][0:64, j * 512 + a:(j + 1) * 512],
                    start=True,
                    stop=True,
                )
                nc.tensor.matmul(
                    ps[:, 512:1024 - a],
                    lhsT=kt_sb[p][64:128, c * 128:(c + 1) * 128],
                    rhs=qt_sb[p][64:128, j * 512 + a:(j + 1) * 512],
                    start=True,
                    stop=True,
                )
                es = epool.tile([128, 1024], BF16, name="es", bufs=4)
                nc.scalar.activation(
                    es[:, a:1024 - a],
                    ps[:, a:1024 - a],
                    mybir.ActivationFunctionType.Exp,
                )
                if a or c == 4 * j:
                    nc.vector.tensor_mul(
                        out=es[:, a:a + 128], in0=es[:, a:a + 128], in1=mask_sb[:, :],
                    )
                    nc.vector.tensor_mul(
                        out=es[:, 512:640], in0=es[:, 512:640], in1=mask_sb[:, :],
                    )
                credit[0] += ((1024 - 2 * a) + 352) / 1.2 - (2 * (512 - a) / 2.4 + 60)
                return es, a

            def emit_pv(p, j, c, nck, es, a, ps_z, ps_s):
                need(f"v{c}")
                for hi in range(2):
                    col = 64 * hi
                    hcore = 2 * p + hi
                    rhs = es[:, a:512] if hi == 0 else es[:, 512:1024 - a]
                    nc.tensor.matmul(
                        ps_z[col:col + 64, a:512],
                        lhsT=v_sb[:, c, hcore * DH:(hcore + 1) * DH],
                        rhs=rhs,
                        start=(c == 0),
                        stop=(c == nck - 1),
                        tile_position=(0, col),
                        skip_group_check=True,
                    )
                for hi in range(2):
                    col = 64 * hi
                    rhs = es[:, a:512] if hi == 0 else es[:, 512:1024 - a]
                    nc.tensor.matmul(
                        ps_s[col:col + 64, a:512],
                        lhsT=ones_bf[:, :],
                        rhs=rhs,
                        start=(c == 0),
                        stop=(c == nck - 1),
                        tile_position=(0, col),
                        skip_group_check=True,
                    )
                credit[0] -= 4 * (512 - a) / 2.4 + 120

            def finish_iter(p, j, ps_z, ps_s):
                recip = spool.tile([128, 512], F32, name="recip")
                nc.vector.reciprocal_approx_fast(recip[:, :], ps_s[:, :])
                zt = ztpool.tile([128, 512], BF16, name="zt")
                nc.vector.tensor_mul(zt[:, :], ps_z[:, :], recip[:, :])
                zts[(p, j)] = zt
                if p == 1:
                    for d in range(DC):
                        filler.append(wo_unit(j, d))

            # flat global chunk pipeline across all (p, j) iterations;
            # scores run one chunk ahead of PV so exp always overlaps.
            # Filler units needed by iteration i+1 are drained evenly
            # across iteration i's chunks (deadline quota); extra units
            # drain on banked ACT-slack credit.
            iters = [(p, j) for p in range(NPAIR) for j in range(QT)]
            stream = []
            for i, (p, j) in enumerate(iters):
                for c in range(4 * (j + 1)):
                    stream.append((i, p, j, c))

            def due_len(i):
                if i + 1 >= len(iters):
                    return 0
                p1, j1 = iters[i + 1]
                tags = {f"q{p1}t{j1}", f"k{p1}t{j1}"}
                tags |= {f"v{k}" for k in range(4 * (j1 + 1))}
                pos = [k for k, (tag, _, _) in enumerate(filler) if tag in tags]
                return (max(pos) + 1) if pos else 0

            prev = None
            it_state = {}
            for g, (i, p, j, c) in enumerate(stream):
                nck = 4 * (j + 1)
                if c == 0:
                    it_state[i] = [due_len(i), drained[0]]  # [due, base]
                es, a = emit_scores(p, j, c)
                st = it_state[i]
                want = -(-st[0] * (c + 1) // nck)  # ceil
                while filler and drained[0] - st[1] < want:
                    drain_one()
                drain_credit()
                if prev is not None:
                    pi, pp, pj, pc, pes, pa, p_z, p_s = prev
                    emit_pv(pp, pj, pc, 4 * (pj + 1), pes, pa, p_z, p_s)
                    if pc == 4 * (pj + 1) - 1:
                        finish_iter(pp, pj, p_z, p_s)
                if c == 0:
                    ps_z = psP.tile([128, 512], F32, name="ps_z", bufs=1)
                    ps_s = psP.tile([128, 512], F32, name="ps_s", bufs=1)
                prev = (i, p, j, c, es, a, ps_z, ps_s)
            pi, pp, pj, pc, pes, pa, p_z, p_s = prev
            emit_pv(pp, pj, pc, 4 * (pj + 1), pes, pa, p_z, p_s)
            finish_iter(pp, pj, p_z, p_s)
            spill_mode[0] = True
            while filler:
                drain_one()

    nc.compile()
    return nc


_cache = {}


def _get(with_bias: bool):
    if with_bias not in _cache:
        _cache[with_bias] = _build(with_bias)
    return _cache[with_bias]


def kernel(x, W_Q, W_K, W_V, W_O, b_Q, b_K, b_V, b_O):
    global LAST_EXEC_NS
    x = np.asarray(x, dtype=np.float32)
    W_Q = np.asarray(W_Q, dtype=np.float32)
    W_K = np.asarray(W_K, dtype=np.float32)
    W_V = np.asarray(W_V, dtype=np.float32)
    W_O = np.asarray(W_O, dtype=np.float32)
    b_Q = np.asarray(b_Q, dtype=np.float32)
    b_K = np.asarray(b_K, dtype=np.float32)
    b_V = np.asarray(b_V, dtype=np.float32)
    b_O = np.asarray(b_O, dtype=np.float32)

    with_bias = bool(np.any(b_Q) or np.any(b_K) or np.any(b_V))
    nc = _get(with_bias)

    xT = np.ascontiguousarray(x.transpose(0, 2, 1))  # [B, DM, S]
    kp = np.arange(128)[:, None]
    qf = np.arange(128)[None, :]
    mask = np.where(qf >= kp, 1.0, 0.0).astype(ml_dtypes.bfloat16)

    in_maps = []
    for core in range(8):
        b, g = divmod(core, 4)
        hs = slice(NH * g, NH * g + NH)
        bf = ml_dtypes.bfloat16

        def packqk(w):  # [DM, NH*DH] -> [128, NPAIR, DC, 128] pair-major
            return np.ascontiguousarray(
                w.reshape(DC, 128, NPAIR, 128).transpose(1, 2, 0, 3)
                .reshape(128, NPAIR * DC * 128)
            )

        def packw(w):  # [DM, NH*DH] -> [128, DC*NH*DH] chunk-major
            return np.ascontiguousarray(
                w.reshape(DC, 128, NH * DH).transpose(1, 0, 2).reshape(128, DC * NH * DH)
            )

        m = {
            "xT": xT[b].astype(bf),
            "wq": packqk((W_Q[hs] * 0.125).transpose(1, 0, 2).reshape(DM, NH * DH).astype(bf)),
            "wk": packqk(W_K[hs].transpose(1, 0, 2).reshape(DM, NH * DH).astype(bf)),
            "wv": packw(W_V[hs].transpose(1, 0, 2).reshape(DM, NH * DH).astype(bf)),
            "wo": np.ascontiguousarray(
                W_O[hs].reshape(NH * DH, DM).astype(bf)
                .reshape(NPAIR, 128, DM).transpose(1, 0, 2).reshape(128, NPAIR * DM)
            ),
            "mask": mask,
        }
        if with_bias:
            m["bq"] = (b_Q[hs] * 0.125).reshape(1, NH * DH).astype(bf)
            m["bk"] = b_K[hs].reshape(1, NH * DH).astype(bf)
            m["bv"] = b_V[hs].reshape(1, NH * DH).astype(bf)
        in_maps.append(m)

    kwargs = {}
    if TRACE:
        kwargs = {"trace": True}
        if TRACE_DIR:
            kwargs["tmpdir"] = TRACE_DIR
    res = run_bass_kernel_spmd(nc, in_maps, list(range(8)), **kwargs)
    LAST_EXEC_NS = res.exec_time_ns

    out = np.empty((B, S, DM), dtype=np.float32)
    for b in range(B):
        acc = res.results[4 * b]["outT"].astype(np.float32)
        for g in range(1, 4):
            acc = acc + res.results[4 * b + g]["outT"].astype(np.float32)
        # [128, DC, S] -> [DM, S]
        full = acc.transpose(1, 0, 2).reshape(DM, S)
        out[b] = full.T + b_O[None, :]
    return out
